# revision 22
# baseline (speedup 1.0000x reference)
"""Trainium2 Bass kernel for nn_MHA_63118839382398.

Full MHA block: fused QKV projection, per-head RMSNorm on q/k, rotate-half
RoPE, causal softmax attention, output projection.

Sharding over 8 NeuronCores: core c handles batch b = c//2 and heads
[8*(c%2), 8*(c%2)+8) (tensor parallel over head halves within a batch
pair). Each core computes a partial out-projection; a 2-rank
ReduceScatter (bf16) over each pair sums the partials and leaves each
core with half of that batch's token rows, which the host reassembles.

Layout strategy (all transposed, feats x tokens), so every matmul
contraction sits on the partition axis with no on-chip transposes except
V (cheap PE-mode 128x128 transposes).

v2 structure (two phases, engine-balance driven):
  P phase: projections + rms + rope for ALL 4 head-pair waves, emitted
    densely so the PE never idles (HAM stays warm). Copies ride the
    otherwise-idle ACT engine; rms/rec broadcasts ride gpsimd
    partition_broadcast.
  A phase: attention in query-chunk-major order (qc outer, wave inner):
    scores for both head halves go to one 2-bank PSUM tile so ONE exp
    instruction covers both (ACT fixed cost ~430ns dominates, so batch).
    Each qc's out-projection + 2-rank ReduceScatter is spread into the
    next qc's attention as PE filler, so collectives overlap compute.
    Epilogue 1/denominator on DVE reciprocal (no ACT Ln/Exp).
"""

import sys

if "/opt/trn_rl_repo" not in sys.path:
    sys.path.insert(0, "/opt/trn_rl_repo")

import numpy as np
import ml_dtypes

import concourse.bass as bass
import concourse.tile as tile
from concourse import bacc, mybir
from concourse.bass_utils import run_bass_kernel_spmd
from concourse.masks import make_identity

# Problem constants (hardcoded per harness contract).
B = 4
N = 2048
D_MODEL = 1024
N_HEADS = 16
D_HEAD = 64
ROPE_BASE = 10000.0
EPS = float(np.finfo(np.float32).eps)
N_CORES = 8

HPC = N_HEADS // 2          # heads per core = 8
WAVES = HPC // 2            # head-pair waves = 4
TOKCH = 512                 # token chunk for projections / q chunks
NT = N // TOKCH             # 4
QT = 128                    # query tile for mask classification
NQT = N // QT               # 16
KB = 128                    # key block
NKB = N // KB               # 16
DC = 128                    # dmodel chunk
NDC = D_MODEL // DC         # 8

F32 = mybir.dt.float32
BF16 = mybir.dt.bfloat16
BF = ml_dtypes.bfloat16

ACT = mybir.ActivationFunctionType

_CACHE = {}


def _pin_act_tables(arch):
    """Steer bacc's ACT-table-set choice to natural_log_exp_and_others.

    The insertion pass picks the first set containing each activation's
    function; removing our functions from every other set's selection
    metadata makes Copy/Square/Ln/Exp resolve to the one set that has
    them all (no mid-kernel table reloads)."""
    from concourse.hw_specs import get_activation_tables

    tables = get_activation_tables(arch)  # cached by reference
    keep = "natural_log_exp_and_others"
    if keep not in tables:
        return
    ours = {ACT.Copy, ACT.Square, ACT.Ln, ACT.Exp, ACT.Identity}
    for name, fns in tables.items():
        if name != keep:
            fns -= ours


def _classify_mask(mask):
    """Per (key-block, query-tile) classification of the mask.

    Returns (state[NKB][NQT], patterns): state is 'skip' (all masked),
    'full' (none masked), or an index into patterns: unique (128,128)
    bf16 0/1 tiles indexed [key, query]."""
    mask = np.asarray(mask)
    assert mask.shape == (N, N)
    patterns = []
    pat_keys = {}
    state = [[None] * NQT for _ in range(NKB)]
    for kb in range(NKB):
        for qt in range(NQT):
            blk = mask[qt * QT : (qt + 1) * QT, kb * KB : (kb + 1) * KB]
            if blk.all():
                state[kb][qt] = "skip"
            elif not blk.any():
                state[kb][qt] = "full"
            else:
                tileq = (~blk.T).astype(BF)
                key = tileq.tobytes()
                if key not in pat_keys:
                    pat_keys[key] = len(patterns)
                    patterns.append(tileq)
                state[kb][qt] = pat_keys[key]
    return state, patterns


def _build_program(state, n_patterns):
    """Build the SPMD Bass program (same graph on all 8 cores)."""
    nc = bacc.Bacc(
        "TRN2", target_bir_lowering=False, debug=False, num_devices=N_CORES
    )
    _pin_act_tables(nc.m.arch)

    p_xt = nc.dram_tensor("xt", [D_MODEL, N], BF16, kind="ExternalInput").ap()
    p_wqk = nc.dram_tensor("wqk", [128, 2, WAVES, NDC, 128], BF16, kind="ExternalInput").ap()
    p_wv = nc.dram_tensor("wv", [128, WAVES, NDC, 128], BF16, kind="ExternalInput").ap()
    p_wo = nc.dram_tensor("wo", [128, 4, D_MODEL], BF16, kind="ExternalInput").ap()
    # rope tables (gain-free, shared by q/k): [128, cos/sin, N]
    p_rope = nc.dram_tensor("rope", [128, 2, N], BF16, kind="ExternalInput").ap()
    p_wcol = nc.dram_tensor("wcol", [128, 2], F32, kind="ExternalInput").ap()
    p_ind2 = nc.dram_tensor("ind2", [128, 2], BF16, kind="ExternalInput").ap()
    p_wfold = nc.dram_tensor("wfold", [2, 128], BF16, kind="ExternalInput").ap()
    p_pswap = nc.dram_tensor("pswap", [128, 128], BF16, kind="ExternalInput").ap()
    if n_patterns:
        p_pat = nc.dram_tensor(
            "pat", [128, n_patterns, 128], BF16, kind="ExternalInput"
        ).ap()
    p_out = nc.dram_tensor("out", [N // 2, D_MODEL], F32, kind="ExternalOutput").ap()

    y_parts = [
        nc.dram_tensor(f"y_part{qc}", [TOKCH, D_MODEL], BF16) for qc in range(NT)
    ]
    rs_outs = [
        nc.dram_tensor(f"rs_out{qc}", [256, D_MODEL], BF16) for qc in range(NT)
    ]
    # tiny dummy collective to absorb the ~11.5us cc-stream spin-up
    cc_warm_in = nc.dram_tensor("ccw_in", [2, 128], BF16)
    cc_warm_out = nc.dram_tensor("ccw_out", [1, 128], BF16)

    QPC = TOKCH // QT  # query tiles per chunk = 4
    n_kb = [0] * NT
    qlo_t = {}
    for qc in range(NT):
        for kb in range(NKB):
            sub = [state[kb][qc * QPC + j] for j in range(QPC)]
            if all(s == "skip" for s in sub):
                continue
            n_kb[qc] = max(n_kb[qc], kb + 1)
            lead = 0
            while sub[lead] == "skip":
                lead += 1
            qlo_t[(qc, kb)] = lead

    with tile.TileContext(nc) as tc:
        import contextlib

        ctx = contextlib.ExitStack()
        with ctx:
            singles = ctx.enter_context(tc.tile_pool(name="singles", bufs=1))
            wavep = ctx.enter_context(tc.tile_pool(name="wavep", bufs=2))
            invp = ctx.enter_context(tc.tile_pool(name="invp", bufs=2))
            work = ctx.enter_context(tc.tile_pool(name="work", bufs=2))
            espool = ctx.enter_context(tc.tile_pool(name="es", bufs=4))
            epi = ctx.enter_context(tc.tile_pool(name="epi", bufs=1))
            outp = ctx.enter_context(tc.tile_pool(name="outp", bufs=2))
            finp = ctx.enter_context(tc.tile_pool(name="finp", bufs=1))

            # PSUM budget (8 banks): ps tag "s" 2x[128,2,512]f32 = 4 banks,
            # po 1x[128,2,512] = 2 banks, pmid 2x[128,512] = 2 banks.
            ps = ctx.enter_context(tc.tile_pool(name="ps", bufs=2, space="PSUM"))
            ppo = ctx.enter_context(tc.tile_pool(name="ppo", bufs=1, space="PSUM"))
            pmid = ctx.enter_context(tc.tile_pool(name="pmid", bufs=2, space="PSUM"))

            # ---- resident constants -------------------------------------
            xt_sb = [singles.tile([128, NDC, TOKCH], BF16, name=f"xt{t}") for t in range(NT)]
            wqk_sb = [
                singles.tile([128, 2, NDC, 128], BF16, name=f"wqk{w}")
                for w in range(WAVES)
            ]
            wv_sb = [
                singles.tile([128, NDC, 128], BF16, name=f"wv{w}")
                for w in range(WAVES)
            ]
            rope_sb = singles.tile([128, 2, N], BF16)
            wcol = singles.tile([128, 2], F32)
            nc.sync.dma_start(out=wcol, in_=p_wcol)
            ident = singles.tile([128, 128], BF16)
            make_identity(nc, ident)
            eps_sb = singles.tile([128, 1], F32)
            nc.vector.memset(eps_sb, EPS)
            pswap = singles.tile([128, 128], BF16)
            nc.sync.dma_start(out=pswap, in_=p_pswap)
            ind2 = singles.tile([128, 2], BF16)
            nc.sync.dma_start(out=ind2, in_=p_ind2)
            wfold = singles.tile([2, 128], BF16)
            nc.sync.dma_start(out=wfold, in_=p_wfold)
            if n_patterns:
                pat_sb = singles.tile([128, n_patterns, 128], BF16)
            yt_sb = singles.tile([128, WAVES, N], BF16)
            wo_sb = singles.tile([128, 4, D_MODEL], BF16)
            # per-wave persistent attention operands
            qk_rot = [
                singles.tile([128, 2, N], BF16, name=f"qkrot{w}")
                for w in range(WAVES)
            ]
            v_sb = [
                singles.tile([128, NKB, 130], BF16, name=f"vsb{w}")
                for w in range(WAVES)
            ]

            # ---- initial DMAs (spread across queues, chunk-granular) ----
            for t in range(NT):
                eng = [nc.sync, nc.scalar, nc.gpsimd, nc.sync][t]
                for dc in range(NDC):
                    eng.dma_start(
                        out=xt_sb[t][:, dc, :],
                        in_=p_xt[dc * DC : (dc + 1) * DC, t * TOKCH : (t + 1) * TOKCH],
                    )
            for w in range(WAVES):
                eng = [nc.scalar, nc.gpsimd, nc.sync, nc.scalar][w]
                eng.dma_start(out=wqk_sb[w], in_=p_wqk[:, :, w, :, :])
                eng.dma_start(out=wv_sb[w], in_=p_wv[:, w, :, :])
            nc.gpsimd.dma_start(out=rope_sb, in_=p_rope)
            if n_patterns:
                nc.gpsimd.dma_start(out=pat_sb, in_=p_pat)
            nc.gpsimd.dma_start(out=wo_sb, in_=p_wo)
            for w in range(WAVES):
                nc.vector.memset(v_sb[w][:, :, 64:65], 1.0)
                nc.vector.memset(v_sb[w][:, :, 129:130], 1.0)
            # collective stream warm-up
            nc.gpsimd.collective_compute(
                "ReduceScatter",
                mybir.AluOpType.add,
                ins=[cc_warm_in.ap().opt()],
                outs=[cc_warm_out.ap().opt()],
                replica_groups=[[0, 1], [2, 3], [4, 5], [6, 7]],
            )

            # =============== P phase: proj + rms + rope ==================
            def emit_P_chunk(w, t):
                tsl = slice(t * TOKCH, (t + 1) * TOKCH)
                # fused q,k projection pair -> one 2-bank psum tile
                pj = ps.tile([128, 2, TOKCH], F32, tag="s", name="pj")
                for qk in range(2):
                    for dc in range(NDC):
                        nc.tensor.matmul(
                            pj[:, qk, :],
                            lhsT=wqk_sb[w][:, qk, dc, :],
                            rhs=xt_sb[t][:, dc, :],
                            start=(dc == 0),
                            stop=(dc == NDC - 1),
                        )
                # psum -> sbuf copies on ACT (idle in P); rms gains ride the
                # per-partition scale of the raw copies
                raw = wavep.tile([128, 2, TOKCH], BF16, tag="raw", name="raw")
                for qk in range(2):
                    nc.scalar.activation(
                        raw[:, qk, :], pj[:, qk, :], ACT.Copy,
                        scale=wcol[:, qk : qk + 1],
                    )
                sq = work.tile([128, 2, TOKCH], BF16, tag="sq")
                nc.scalar.square(sq, pj)          # ACT (pre-gain squares)
                lnm = work.tile([2, 2, TOKCH], BF16, tag="lnm")
                inv = invp.tile([2, 2, TOKCH], BF16, tag="inv", name="inv")
                for qk in range(2):
                    ssp = pmid.tile([2, TOKCH], F32, tag="m", name="ssp")
                    nc.tensor.matmul(
                        ssp, lhsT=ind2, rhs=sq[:, qk, :], start=True, stop=True
                    )
                    nc.scalar.activation(
                        lnm[:, qk, :], ssp, ACT.Ln,
                        bias=eps_sb[0:2, :], scale=1.0 / D_HEAD,
                    )
                    nc.scalar.activation(
                        inv[:, qk, :], lnm[:, qk, :], ACT.Exp, scale=-0.5
                    )
                # V projection + transpose into (keys x dims) layout
                pjv = pmid.tile([128, TOKCH], F32, tag="m", name="pjv")
                for dc in range(NDC):
                    nc.tensor.matmul(
                        pjv,
                        lhsT=wv_sb[w][:, dc, :],
                        rhs=xt_sb[t][:, dc, :],
                        start=(dc == 0),
                        stop=(dc == NDC - 1),
                    )
                vt = work.tile([128, TOKCH], BF16, tag="vt")
                nc.scalar.copy(vt, pjv)           # ACT
                for sv2 in range(2):
                    ptr = pmid.tile([128, 2, 128], BF16, tag="m", name="ptr")
                    for s2 in range(2):
                        sview = sv2 * 2 + s2
                        nc.tensor.transpose(
                            ptr[:, s2, :],
                            vt[:, sview * 128 : (sview + 1) * 128],
                            ident,
                        )
                    kb0 = t * 4 + sv2 * 2
                    nc.vector.tensor_copy(
                        v_sb[w][:, kb0 : kb0 + 2, 0:64], ptr[:, :, 0:64]
                    )
                    nc.vector.tensor_copy(
                        v_sb[w][:, kb0 : kb0 + 2, 65:129], ptr[:, :, 64:128]
                    )
                # rms factors broadcast 2 rows -> 128 rows (PE indicator matmul)
                fac = ps.tile([128, 2, TOKCH], F32, tag="s", name="fac")
                for qk in range(2):
                    nc.tensor.matmul(
                        fac[:, qk, :], lhsT=wfold, rhs=inv[:, qk, :],
                        start=True, stop=True,
                    )
                qn = work.tile([128, 2, TOKCH], BF16, tag="qn")
                nc.vector.tensor_mul(qn, raw, fac)
                swp = ps.tile([128, 2, TOKCH], F32, tag="s", name="swp")
                for qk in range(2):
                    nc.tensor.matmul(
                        swp[:, qk, :], lhsT=pswap, rhs=qn[:, qk, :],
                        start=True, stop=True,
                    )
                qcos = work.tile([128, 2, TOKCH], BF16, tag="qcos")
                qsin = work.tile([128, 2, TOKCH], BF16, tag="qsin")
                for qk in range(2):
                    nc.vector.tensor_mul(
                        qcos[:, qk, :], qn[:, qk, :], rope_sb[:, 0, tsl]
                    )
                    nc.vector.tensor_mul(
                        qsin[:, qk, :], swp[:, qk, :], rope_sb[:, 1, tsl]
                    )
                nc.vector.tensor_add(qk_rot[w][:, :, tsl], qcos, qsin)

            # =============== A phase: attention, qc-major ================
            def emit_D(qc, w, prologue):
                """Attention for (qc, w). `prologue` is a list of closures
                (previous wave's epilogue, out-proj filler units) emitted
                between the first score pairs and the first PV so the PE
                FIFO never stalls on their dependencies. Returns this
                wave's epilogue closure."""
                kbs = [kb for kb in range(n_kb[qc]) if (qc, kb) in qlo_t]
                po = ppo.tile([128, 2, TOKCH], F32, tag="po", name="po")
                first = [True, True]
                pend = []

                def flush_pv(kb, es, last):
                    qlo = qlo_t[(qc, kb)] * QT
                    osl = slice(qlo, TOKCH)
                    for h2 in range(2):
                        nc.tensor.matmul(
                            po[0:65, h2, osl],
                            lhsT=v_sb[w][:, kb, 65 * h2 : 65 * h2 + 65],
                            rhs=es[:, h2, osl],
                            start=first[h2],
                            stop=last,
                        )
                        first[h2] = False

                for i, kb in enumerate(kbs):
                    qlo = qlo_t[(qc, kb)] * QT
                    csl = slice(qc * TOKCH + qlo, (qc + 1) * TOKCH)
                    osl = slice(qlo, TOKCH)
                    pst = ps.tile([128, 2, TOKCH], F32, tag="s", name="pst")
                    for h2 in range(2):
                        hr = slice(64 * h2, 64 * h2 + 64)
                        nc.tensor.matmul(
                            pst[:, h2, osl],
                            lhsT=qk_rot[w][hr, 1, kb * KB : (kb + 1) * KB],
                            rhs=qk_rot[w][hr, 0, csl],
                            start=True,
                            stop=True,
                        )
                    es = espool.tile([128, 2, TOKCH], BF16, tag="es", name="es")
                    nc.scalar.activation(
                        es[:, :, osl], pst[:, :, osl], ACT.Exp,
                        scale=float(D_HEAD) ** -0.5,
                    )
                    for j in range(qlo // QT, QPC):
                        st = state[kb][qc * QPC + j]
                        if isinstance(st, int):
                            jsl = slice(j * QT, (j + 1) * QT)
                            for h2 in range(2):
                                nc.vector.tensor_mul(
                                    es[:, h2, jsl], es[:, h2, jsl],
                                    pat_sb[:, st, :],
                                )
                    if i == 0 and prologue:
                        for fn in prologue:
                            fn()
                        prologue = []
                    pend.append((kb, es))
                    if len(pend) > 2:
                        k0, e0 = pend.pop(0)
                        flush_pv(k0, e0, False)
                for i, (k0, e0) in enumerate(pend):
                    flush_pv(k0, e0, i == len(pend) - 1)

                def epilogue():
                    # po rows 0:63 = y_raw, row 64 = softmax denominator
                    yr = epi.tile([65, 2, TOKCH], BF16, tag="yr", name="yr")
                    nc.vector.tensor_copy(yr, po[0:65, :, :])
                    rec = epi.tile([1, 2, TOKCH], F32, tag="rec", name="rec")
                    nc.vector.reciprocal(rec, po[64:65, :, :])
                    recb = epi.tile([1, 2, TOKCH], BF16, tag="recb", name="recb")
                    nc.vector.tensor_copy(recb, rec)
                    f2 = epi.tile([64, 2, TOKCH], BF16, tag="f2", name="f2")
                    for h2 in range(2):
                        nc.gpsimd.partition_broadcast(
                            f2[:, h2, :], recb[:, h2, :]
                        )
                    for h2 in range(2):
                        nc.vector.tensor_mul(
                            yt_sb[
                                64 * h2 : 64 * h2 + 64, w,
                                qc * TOKCH : (qc + 1) * TOKCH,
                            ],
                            yr[0:64, h2, :],
                            f2[:, h2, :],
                        )

                return epilogue

            def out_unit(qc, i):
                """One eighth of qc's out-projection: 128 tokens x 512
                out-features -> y_parts[qc]."""
                def fn():
                    t2 = qc * 4 + i // 2
                    ec = i % 2
                    pot = pmid.tile([128, TOKCH], F32, tag="m", name="pot")
                    for fc in range(4):
                        nc.tensor.matmul(
                            pot,
                            lhsT=yt_sb[:, fc, t2 * 128 : (t2 + 1) * 128],
                            rhs=wo_sb[:, fc, ec * TOKCH : (ec + 1) * TOKCH],
                            start=(fc == 0),
                            stop=(fc == 3),
                        )
                    osb = outp.tile([128, TOKCH], BF16, tag="o", name="osb")
                    nc.vector.tensor_copy(osb, pot)
                    r2 = t2 * 128 - qc * TOKCH
                    nc.sync.dma_start(
                        out=y_parts[qc].ap()[
                            r2 : r2 + 128, ec * TOKCH : (ec + 1) * TOKCH
                        ],
                        in_=osb,
                    )
                return fn

            def out_finish(qc):
                def fn():
                    nc.gpsimd.collective_compute(
                        "ReduceScatter",
                        mybir.AluOpType.add,
                        ins=[y_parts[qc].ap().opt()],
                        outs=[rs_outs[qc].ap().opt()],
                        replica_groups=[[0, 1], [2, 3], [4, 5], [6, 7]],
                    )
                    for half2 in range(2):
                        for ec2 in range(2):
                            rt = finp.tile([128, TOKCH], BF16, tag="rt", name="rt")
                            nc.gpsimd.dma_start(
                                out=rt,
                                in_=rs_outs[qc].ap()[
                                    half2 * 128 : (half2 + 1) * 128,
                                    ec2 * TOKCH : (ec2 + 1) * TOKCH,
                                ],
                            )
                            ro = finp.tile([128, TOKCH], F32, tag="ro", name="ro")
                            nc.vector.tensor_copy(ro, rt)
                            r0 = qc * 256 + half2 * 128
                            nc.gpsimd.dma_start(
                                out=p_out[
                                    r0 : r0 + 128, ec2 * TOKCH : (ec2 + 1) * TOKCH
                                ],
                                in_=ro,
                            )
                return fn

            # ---------------- emission schedule --------------------------
            for w in range(WAVES):
                for t in range(NT):
                    emit_P_chunk(w, t)

            filler = []   # deferred out-proj units from the previous qc
            prev_epi = None
            for qc in range(NT):
                for w in range(WAVES):
                    prologue = []
                    if prev_epi is not None:
                        prologue.append(prev_epi)
                        prev_epi = None
                    # spread up to 3 out-proj filler units per wave
                    for _ in range(3):
                        if filler:
                            prologue.append(filler.pop(0))
                    prev_epi = emit_D(qc, w, prologue)
                # epilogue of wave 3 must land before qc's out-proj units
                assert not filler
                filler = [out_unit(qc, i) for i in range(8)] + [out_finish(qc)]
                if qc == NT - 1:
                    prev_epi()
                    for fn in filler:
                        fn()
                    filler = []

    nc.compile()
    return nc


def _host_prep(x, mask, pos, W_qkv, W_out, qn_w, kn_w):
    x = np.asarray(x, dtype=np.float32)
    mask = np.asarray(mask)
    pos = np.asarray(pos).astype(np.float64)
    W_qkv = np.asarray(W_qkv, dtype=np.float32)
    W_out = np.asarray(W_out, dtype=np.float32)
    qn_w = np.asarray(qn_w, dtype=np.float32)
    kn_w = np.asarray(kn_w, dtype=np.float32)

    inv_freq = 1.0 / (ROPE_BASE ** (np.arange(0, D_HEAD, 2, dtype=np.float64) / D_HEAD))
    ang = pos[:, None] * inv_freq[None, :]  # (N, 32)
    cosT = np.cos(ang).T.astype(np.float32)  # (32, N)
    sinT = np.sin(ang).T.astype(np.float32)

    # gain-free tables shared by q and k (gains applied via per-partition
    # ACT scale on the raw copies)
    cos_d = np.tile(cosT, (4, 1))
    sin_d = np.tile(np.concatenate([-sinT, sinT], axis=0), (2, 1))
    rope = np.stack([cos_d, sin_d], axis=1).astype(BF)  # (128, 2, N)
    wcol_np = np.stack([np.tile(qn_w, 2), np.tile(kn_w, 2)], axis=1).astype(
        np.float32
    )  # (128, 2)

    pswap_np = np.zeros((128, 128), dtype=np.float32)
    for a in range(2):
        for r in range(32):
            pswap_np[64 * a + r, 64 * a + 32 + r] = 1.0
            pswap_np[64 * a + 32 + r, 64 * a + r] = 1.0
    pswap_np = pswap_np.astype(BF)

    ind2_np = np.zeros((128, 2), dtype=np.float32)
    ind2_np[0:64, 0] = 1.0
    ind2_np[64:128, 1] = 1.0
    ind2_np = ind2_np.astype(BF)
    wfold_np = np.ascontiguousarray(ind2_np.T)  # (2, 128)

    state, patterns = _classify_mask(mask)
    if patterns:
        pat = np.stack(patterns, axis=1).astype(BF)
    else:
        pat = None

    q_rows = lambda h: slice(h * 192, h * 192 + 64)
    k_rows = lambda h: slice(h * 192 + 64, h * 192 + 128)
    v_rows = lambda h: slice(h * 192 + 128, h * 192 + 192)

    in_maps = []
    for c in range(N_CORES):
        b, half = divmod(c, 2)
        hs = [8 * half + i for i in range(8)]
        wqk = np.concatenate(
            [W_qkv[q_rows(h)] for h in hs] + [W_qkv[k_rows(h)] for h in hs], axis=0
        ).T  # (1024 dmodel, 1024 cols)
        wv = np.concatenate([W_qkv[v_rows(h)] for h in hs], axis=0).T
        wo = W_out[:, 512 * half : 512 * half + 512].T  # (512, 1024)
        # (128, 2, WAVES, NDC, 128): [p, qk, w, dc, f]
        wqk_re = np.ascontiguousarray(
            wqk.reshape(NDC, 128, 2, WAVES, 128).transpose(1, 2, 3, 0, 4)
        )
        wv_re = np.ascontiguousarray(
            wv.reshape(NDC, 128, WAVES, 128).transpose(1, 2, 0, 3)
        )
        wo_re = np.ascontiguousarray(wo.reshape(4, 128, 1024).transpose(1, 0, 2))
        m = {
            "xt": np.ascontiguousarray(x[b].T).astype(BF),
            "wqk": wqk_re.astype(BF),
            "wv": wv_re.astype(BF),
            "wo": wo_re.astype(BF),
            "rope": rope,
            "wcol": wcol_np,
            "ind2": ind2_np,
            "wfold": wfold_np,
            "pswap": pswap_np,
        }
        if pat is not None:
            m["pat"] = pat
        in_maps.append(m)
    return in_maps, state, (0 if pat is None else pat.shape[1])


def kernel(x, mask, pos, W_qkv, W_out, qn_w, kn_w, _trace=False):
    in_maps, state, n_pat = _host_prep(x, mask, pos, W_qkv, W_out, qn_w, kn_w)
    key = (str(state), n_pat)
    if key not in _CACHE:
        _CACHE[key] = _build_program(state, n_pat)
    nc = _CACHE[key]
    res = run_bass_kernel_spmd(nc, in_maps, list(range(N_CORES)), trace=_trace)
    out = np.empty((B, N, D_MODEL), dtype=np.float32)
    for b in range(B):
        lo = res.results[2 * b]["out"]
        hi = res.results[2 * b + 1]["out"]
        for qc in range(NT):
            out[b, qc * TOKCH : qc * TOKCH + 256] = lo[qc * 256 : (qc + 1) * 256]
            out[b, qc * TOKCH + 256 : (qc + 1) * TOKCH] = hi[qc * 256 : (qc + 1) * 256]
    kernel._last_results = res
    return out


# revision 24
# speedup vs baseline: 1.0439x; 1.0439x over previous
"""Trainium2 Bass kernel for nn_MHA_63118839382398.

Full MHA block: fused QKV projection, per-head RMSNorm on q/k, rotate-half
RoPE, causal softmax attention, output projection.

Sharding over 8 NeuronCores: core c handles batch b = c//2 and heads
[8*(c%2), 8*(c%2)+8) (tensor parallel over head halves within a batch
pair). Each core computes a partial out-projection; a 2-rank
ReduceScatter (bf16) over each pair sums the partials and leaves each
core with half of that batch's token rows, which the host reassembles.

Layout strategy (all transposed, feats x tokens), so every matmul
contraction sits on the partition axis with no on-chip transposes except
V (cheap PE-mode 128x128 transposes).

v2 structure (two phases, engine-balance driven):
  P phase: projections + rms + rope for ALL 4 head-pair waves, emitted
    densely so the PE never idles (HAM stays warm). Copies ride the
    otherwise-idle ACT engine; rms/rec broadcasts ride gpsimd
    partition_broadcast.
  A phase: attention in query-chunk-major order (qc outer, wave inner):
    scores for both head halves go to one 2-bank PSUM tile so ONE exp
    instruction covers both (ACT fixed cost ~430ns dominates, so batch).
    Each qc's out-projection + 2-rank ReduceScatter is spread into the
    next qc's attention as PE filler, so collectives overlap compute.
    Epilogue 1/denominator on DVE reciprocal (no ACT Ln/Exp).
"""

import sys

if "/opt/trn_rl_repo" not in sys.path:
    sys.path.insert(0, "/opt/trn_rl_repo")

import numpy as np
import ml_dtypes

import concourse.bass as bass
import concourse.tile as tile
from concourse import bacc, mybir
from concourse.bass_utils import run_bass_kernel_spmd
from concourse.masks import make_identity

# Problem constants (hardcoded per harness contract).
B = 4
N = 2048
D_MODEL = 1024
N_HEADS = 16
D_HEAD = 64
ROPE_BASE = 10000.0
EPS = float(np.finfo(np.float32).eps)
N_CORES = 8

HPC = N_HEADS // 2          # heads per core = 8
WAVES = HPC // 2            # head-pair waves = 4
TOKCH = 512                 # token chunk for projections / q chunks
NT = N // TOKCH             # 4
QT = 128                    # query tile for mask classification
NQT = N // QT               # 16
KB = 128                    # key block
NKB = N // KB               # 16
DC = 128                    # dmodel chunk
NDC = D_MODEL // DC         # 8

F32 = mybir.dt.float32
BF16 = mybir.dt.bfloat16
BF = ml_dtypes.bfloat16

ACT = mybir.ActivationFunctionType

_CACHE = {}


def _pin_act_tables(arch):
    """Steer bacc's ACT-table-set choice to natural_log_exp_and_others.

    The insertion pass picks the first set containing each activation's
    function; removing our functions from every other set's selection
    metadata makes Copy/Square/Ln/Exp resolve to the one set that has
    them all (no mid-kernel table reloads)."""
    from concourse.hw_specs import get_activation_tables

    tables = get_activation_tables(arch)  # cached by reference
    keep = "natural_log_exp_and_others"
    if keep not in tables:
        return
    ours = {ACT.Copy, ACT.Square, ACT.Ln, ACT.Exp, ACT.Identity}
    for name, fns in tables.items():
        if name != keep:
            fns -= ours


def _classify_mask(mask):
    """Per (key-block, query-tile) classification of the mask.

    Returns (state[NKB][NQT], patterns): state is 'skip' (all masked),
    'full' (none masked), or an index into patterns: unique (128,128)
    bf16 0/1 tiles indexed [key, query]."""
    mask = np.asarray(mask)
    assert mask.shape == (N, N)
    patterns = []
    pat_keys = {}
    state = [[None] * NQT for _ in range(NKB)]
    for kb in range(NKB):
        for qt in range(NQT):
            blk = mask[qt * QT : (qt + 1) * QT, kb * KB : (kb + 1) * KB]
            if blk.all():
                state[kb][qt] = "skip"
            elif not blk.any():
                state[kb][qt] = "full"
            else:
                tileq = (~blk.T).astype(BF)
                key = tileq.tobytes()
                if key not in pat_keys:
                    pat_keys[key] = len(patterns)
                    patterns.append(tileq)
                state[kb][qt] = pat_keys[key]
    return state, patterns


def _build_program(state, n_patterns):
    """Build the SPMD Bass program (same graph on all 8 cores)."""
    nc = bacc.Bacc(
        "TRN2", target_bir_lowering=False, debug=False, num_devices=N_CORES
    )
    _pin_act_tables(nc.m.arch)

    p_xt = nc.dram_tensor("xt", [D_MODEL, N], BF16, kind="ExternalInput").ap()
    p_wqk = nc.dram_tensor("wqk", [128, 2, WAVES, NDC, 128], BF16, kind="ExternalInput").ap()
    p_wv = nc.dram_tensor("wv", [128, WAVES, NDC, 128], BF16, kind="ExternalInput").ap()
    p_wo = nc.dram_tensor("wo", [128, 4, D_MODEL], BF16, kind="ExternalInput").ap()
    # rope tables (gain-free, shared by q/k): [128, cos/sin, N]
    p_rope = nc.dram_tensor("rope", [128, 2, N], BF16, kind="ExternalInput").ap()
    p_wcol = nc.dram_tensor("wcol", [128, 2], F32, kind="ExternalInput").ap()
    p_ind2 = nc.dram_tensor("ind2", [128, 2], BF16, kind="ExternalInput").ap()
    p_wfold = nc.dram_tensor("wfold", [2, 128], BF16, kind="ExternalInput").ap()
    p_pswap = nc.dram_tensor("pswap", [128, 128], BF16, kind="ExternalInput").ap()
    if n_patterns:
        p_pat = nc.dram_tensor(
            "pat", [128, n_patterns, 128], BF16, kind="ExternalInput"
        ).ap()
    p_out = nc.dram_tensor("out", [N // 2, D_MODEL], F32, kind="ExternalOutput").ap()

    y_parts = [
        nc.dram_tensor(f"y_part{qc}", [TOKCH, D_MODEL], BF16) for qc in range(NT)
    ]
    rs_outs = [
        nc.dram_tensor(f"rs_out{qc}", [256, D_MODEL], BF16) for qc in range(NT)
    ]
    # tiny dummy collective to absorb the ~11.5us cc-stream spin-up
    cc_warm_in = nc.dram_tensor("ccw_in", [2, 128], BF16)
    cc_warm_out = nc.dram_tensor("ccw_out", [1, 128], BF16)

    QPC = TOKCH // QT  # query tiles per chunk = 4
    n_kb = [0] * NT
    qlo_t = {}
    for qc in range(NT):
        for kb in range(NKB):
            sub = [state[kb][qc * QPC + j] for j in range(QPC)]
            if all(s == "skip" for s in sub):
                continue
            n_kb[qc] = max(n_kb[qc], kb + 1)
            lead = 0
            while sub[lead] == "skip":
                lead += 1
            qlo_t[(qc, kb)] = lead

    with tile.TileContext(nc) as tc:
        import contextlib

        ctx = contextlib.ExitStack()
        with ctx:
            singles = ctx.enter_context(tc.tile_pool(name="singles", bufs=1))
            wavep = ctx.enter_context(tc.tile_pool(name="wavep", bufs=2))
            invp = ctx.enter_context(tc.tile_pool(name="invp", bufs=2))
            work = ctx.enter_context(tc.tile_pool(name="work", bufs=2))
            espool = ctx.enter_context(tc.tile_pool(name="es", bufs=4))
            epi = ctx.enter_context(tc.tile_pool(name="epi", bufs=1))
            outp = ctx.enter_context(tc.tile_pool(name="outp", bufs=2))
            finp = ctx.enter_context(tc.tile_pool(name="finp", bufs=1))

            # PSUM budget (8 banks): ps tag "s" 2x[128,2,512]f32 = 4 banks,
            # po 1x[128,2,512] = 2 banks, pmid 2x[128,512] = 2 banks.
            ps = ctx.enter_context(tc.tile_pool(name="ps", bufs=2, space="PSUM"))
            ppo = ctx.enter_context(tc.tile_pool(name="ppo", bufs=1, space="PSUM"))
            pmid = ctx.enter_context(tc.tile_pool(name="pmid", bufs=2, space="PSUM"))

            # ---- resident constants -------------------------------------
            xt_sb = [singles.tile([128, NDC, TOKCH], BF16, name=f"xt{t}") for t in range(NT)]
            wqk_sb = [
                singles.tile([128, 2, NDC, 128], BF16, name=f"wqk{w}")
                for w in range(WAVES)
            ]
            wv_sb = [
                singles.tile([128, NDC, 128], BF16, name=f"wv{w}")
                for w in range(WAVES)
            ]
            rope_sb = singles.tile([128, 2, N], BF16)
            wcol = singles.tile([128, 2], F32)
            nc.sync.dma_start(out=wcol, in_=p_wcol)
            ident = singles.tile([128, 128], BF16)
            make_identity(nc, ident)
            eps_sb = singles.tile([128, 1], F32)
            nc.vector.memset(eps_sb, EPS)
            pswap = singles.tile([128, 128], BF16)
            nc.sync.dma_start(out=pswap, in_=p_pswap)
            ind2 = singles.tile([128, 2], BF16)
            nc.sync.dma_start(out=ind2, in_=p_ind2)
            wfold = singles.tile([2, 128], BF16)
            nc.sync.dma_start(out=wfold, in_=p_wfold)
            if n_patterns:
                pat_sb = singles.tile([128, n_patterns, 128], BF16)
            yt_sb = singles.tile([128, WAVES, N], BF16)
            wo_sb = singles.tile([128, 4, D_MODEL], BF16)
            # per-wave persistent attention operands
            qk_rot = [
                singles.tile([128, 2, N], BF16, name=f"qkrot{w}")
                for w in range(WAVES)
            ]
            v_sb = [
                singles.tile([128, NKB, 130], BF16, name=f"vsb{w}")
                for w in range(WAVES)
            ]

            # ---- initial DMAs: first-needed first on each queue ----------
            # sync: w0 weights + x chunk 0 (the critical path), then chunk 3
            nc.sync.dma_start(out=wqk_sb[0], in_=p_wqk[:, :, 0, :, :])
            nc.sync.dma_start(out=wv_sb[0], in_=p_wv[:, 0, :, :])
            for dc in range(NDC):
                nc.sync.dma_start(
                    out=xt_sb[0][:, dc, :], in_=p_xt[dc * DC : (dc + 1) * DC, 0:TOKCH]
                )
            # scalar: x chunk 1, w1 weights, x chunk 3
            for dc in range(NDC):
                nc.scalar.dma_start(
                    out=xt_sb[1][:, dc, :],
                    in_=p_xt[dc * DC : (dc + 1) * DC, TOKCH : 2 * TOKCH],
                )
            nc.scalar.dma_start(out=wqk_sb[1], in_=p_wqk[:, :, 1, :, :])
            for dc in range(NDC):
                nc.scalar.dma_start(
                    out=xt_sb[3][:, dc, :],
                    in_=p_xt[dc * DC : (dc + 1) * DC, 3 * TOKCH : 4 * TOKCH],
                )
            nc.scalar.dma_start(out=wqk_sb[3], in_=p_wqk[:, :, 3, :, :])
            nc.scalar.dma_start(out=wv_sb[3], in_=p_wv[:, 3, :, :])
            # gpsimd: rope + x chunk 2 + remaining weights + tables
            nc.gpsimd.dma_start(out=rope_sb, in_=p_rope)
            for dc in range(NDC):
                nc.gpsimd.dma_start(
                    out=xt_sb[2][:, dc, :],
                    in_=p_xt[dc * DC : (dc + 1) * DC, 2 * TOKCH : 3 * TOKCH],
                )
            nc.gpsimd.dma_start(out=wv_sb[1], in_=p_wv[:, 1, :, :])
            nc.gpsimd.dma_start(out=wqk_sb[2], in_=p_wqk[:, :, 2, :, :])
            nc.gpsimd.dma_start(out=wv_sb[2], in_=p_wv[:, 2, :, :])
            if n_patterns:
                nc.gpsimd.dma_start(out=pat_sb, in_=p_pat)
            nc.gpsimd.dma_start(out=wo_sb, in_=p_wo)
            for w in range(WAVES):
                nc.vector.memset(v_sb[w][:, :, 64:65], 1.0)
                nc.vector.memset(v_sb[w][:, :, 129:130], 1.0)
            # collective stream warm-up
            nc.gpsimd.collective_compute(
                "ReduceScatter",
                mybir.AluOpType.add,
                ins=[cc_warm_in.ap().opt()],
                outs=[cc_warm_out.ap().opt()],
                replica_groups=[[0, 1], [2, 3], [4, 5], [6, 7]],
            )

            # =============== P phase: proj + rms + rope ==================
            def emit_P_chunk(w, t):
                tsl = slice(t * TOKCH, (t + 1) * TOKCH)
                # fused q,k projection pair -> one 2-bank psum tile
                pj = ps.tile([128, 2, TOKCH], F32, tag="s", name="pj")
                for qk in range(2):
                    for dc in range(NDC):
                        nc.tensor.matmul(
                            pj[:, qk, :],
                            lhsT=wqk_sb[w][:, qk, dc, :],
                            rhs=xt_sb[t][:, dc, :],
                            start=(dc == 0),
                            stop=(dc == NDC - 1),
                        )
                # psum -> sbuf copies on ACT (idle in P); rms gains ride the
                # per-partition scale of the raw copies
                raw = wavep.tile([128, 2, TOKCH], BF16, tag="raw", name="raw")
                for qk in range(2):
                    nc.scalar.activation(
                        raw[:, qk, :], pj[:, qk, :], ACT.Copy,
                        scale=wcol[:, qk : qk + 1],
                    )
                sq = work.tile([128, 2, TOKCH], BF16, tag="sq")
                nc.scalar.square(sq, pj)          # ACT (pre-gain squares)
                lnm = work.tile([2, 2, TOKCH], BF16, tag="lnm")
                inv = invp.tile([2, 2, TOKCH], BF16, tag="inv", name="inv")
                for qk in range(2):
                    ssp = pmid.tile([2, TOKCH], F32, tag="m", name="ssp")
                    nc.tensor.matmul(
                        ssp, lhsT=ind2, rhs=sq[:, qk, :], start=True, stop=True
                    )
                    nc.scalar.activation(
                        lnm[:, qk, :], ssp, ACT.Ln,
                        bias=eps_sb[0:2, :], scale=1.0 / D_HEAD,
                    )
                    nc.scalar.activation(
                        inv[:, qk, :], lnm[:, qk, :], ACT.Exp, scale=-0.5
                    )
                # V projection + transpose into (keys x dims) layout
                pjv = pmid.tile([128, TOKCH], F32, tag="m", name="pjv")
                for dc in range(NDC):
                    nc.tensor.matmul(
                        pjv,
                        lhsT=wv_sb[w][:, dc, :],
                        rhs=xt_sb[t][:, dc, :],
                        start=(dc == 0),
                        stop=(dc == NDC - 1),
                    )
                vt = work.tile([128, TOKCH], BF16, tag="vt")
                nc.scalar.copy(vt, pjv)           # ACT
                for sv2 in range(2):
                    ptr = pmid.tile([128, 2, 128], BF16, tag="m", name="ptr")
                    for s2 in range(2):
                        sview = sv2 * 2 + s2
                        nc.tensor.transpose(
                            ptr[:, s2, :],
                            vt[:, sview * 128 : (sview + 1) * 128],
                            ident,
                        )
                    kb0 = t * 4 + sv2 * 2
                    nc.vector.tensor_copy(
                        v_sb[w][:, kb0 : kb0 + 2, 0:64], ptr[:, :, 0:64]
                    )
                    nc.vector.tensor_copy(
                        v_sb[w][:, kb0 : kb0 + 2, 65:129], ptr[:, :, 64:128]
                    )
                # rms factors broadcast 2 rows -> 128 rows (PE indicator matmul)
                fac = ps.tile([128, 2, TOKCH], F32, tag="s", name="fac")
                for qk in range(2):
                    nc.tensor.matmul(
                        fac[:, qk, :], lhsT=wfold, rhs=inv[:, qk, :],
                        start=True, stop=True,
                    )
                qn = work.tile([128, 2, TOKCH], BF16, tag="qn")
                nc.vector.tensor_mul(qn, raw, fac)
                swp = ps.tile([128, 2, TOKCH], F32, tag="s", name="swp")
                for qk in range(2):
                    nc.tensor.matmul(
                        swp[:, qk, :], lhsT=pswap, rhs=qn[:, qk, :],
                        start=True, stop=True,
                    )
                qcos = work.tile([128, 2, TOKCH], BF16, tag="qcos")
                qsin = work.tile([128, 2, TOKCH], BF16, tag="qsin")
                for qk in range(2):
                    nc.vector.tensor_mul(
                        qcos[:, qk, :], qn[:, qk, :], rope_sb[:, 0, tsl]
                    )
                    nc.vector.tensor_mul(
                        qsin[:, qk, :], swp[:, qk, :], rope_sb[:, 1, tsl]
                    )
                nc.vector.tensor_add(qk_rot[w][:, :, tsl], qcos, qsin)

            # =============== A phase: attention, qc-major ================
            def emit_D(qc, w, prologue):
                """Attention for (qc, w). `prologue` is a list of closures
                (previous wave's epilogue, out-proj filler units) emitted
                between the first score pairs and the first PV so the PE
                FIFO never stalls on their dependencies. Returns this
                wave's epilogue closure."""
                kbs = [kb for kb in range(n_kb[qc]) if (qc, kb) in qlo_t]
                po = ppo.tile([128, 2, TOKCH], F32, tag="po", name="po")
                first = [True, True]
                pend = []

                def flush_pv(kb, es, last):
                    qlo = qlo_t[(qc, kb)] * QT
                    osl = slice(qlo, TOKCH)
                    for h2 in range(2):
                        nc.tensor.matmul(
                            po[0:65, h2, osl],
                            lhsT=v_sb[w][:, kb, 65 * h2 : 65 * h2 + 65],
                            rhs=es[:, h2, osl],
                            start=first[h2],
                            stop=last,
                        )
                        first[h2] = False

                for i, kb in enumerate(kbs):
                    qlo = qlo_t[(qc, kb)] * QT
                    csl = slice(qc * TOKCH + qlo, (qc + 1) * TOKCH)
                    osl = slice(qlo, TOKCH)
                    pst = ps.tile([128, 2, TOKCH], F32, tag="s", name="pst")
                    for h2 in range(2):
                        hr = slice(64 * h2, 64 * h2 + 64)
                        nc.tensor.matmul(
                            pst[:, h2, osl],
                            lhsT=qk_rot[w][hr, 1, kb * KB : (kb + 1) * KB],
                            rhs=qk_rot[w][hr, 0, csl],
                            start=True,
                            stop=True,
                        )
                    es = espool.tile([128, 2, TOKCH], BF16, tag="es", name="es")
                    nc.scalar.activation(
                        es[:, :, osl], pst[:, :, osl], ACT.Exp,
                        scale=float(D_HEAD) ** -0.5,
                    )
                    for j in range(qlo // QT, QPC):
                        st = state[kb][qc * QPC + j]
                        if isinstance(st, int):
                            jsl = slice(j * QT, (j + 1) * QT)
                            for h2 in range(2):
                                nc.vector.tensor_mul(
                                    es[:, h2, jsl], es[:, h2, jsl],
                                    pat_sb[:, st, :],
                                )
                    if i == 0 and prologue:
                        for fn in prologue:
                            fn()
                        prologue = []
                    pend.append((kb, es))
                    if len(pend) > 2:
                        k0, e0 = pend.pop(0)
                        flush_pv(k0, e0, False)
                for i, (k0, e0) in enumerate(pend):
                    flush_pv(k0, e0, i == len(pend) - 1)

                def epilogue():
                    # po rows 0:63 = y_raw, row 64 = softmax denominator;
                    # 1/den = exp(-ln(den)) on ACT (single-partition DVE
                    # reciprocal measured 6.5us -- ACT is flat ~650ns)
                    yr = epi.tile([64, 2, TOKCH], BF16, tag="yr", name="yr")
                    nc.vector.tensor_copy(yr, po[0:64, :, :])
                    lnd = epi.tile([1, 2, TOKCH], F32, tag="lnd", name="lnd")
                    nc.scalar.activation(lnd, po[64:65, :, :], ACT.Ln)
                    recb = epi.tile([1, 2, TOKCH], BF16, tag="recb", name="recb")
                    nc.scalar.activation(recb, lnd, ACT.Exp, scale=-1.0)
                    f2 = epi.tile([64, 2, TOKCH], BF16, tag="f2", name="f2")
                    for h2 in range(2):
                        nc.gpsimd.partition_broadcast(
                            f2[:, h2, :], recb[:, h2, :]
                        )
                    for h2 in range(2):
                        nc.vector.tensor_mul(
                            yt_sb[
                                64 * h2 : 64 * h2 + 64, w,
                                qc * TOKCH : (qc + 1) * TOKCH,
                            ],
                            yr[:, h2, :],
                            f2[:, h2, :],
                        )

                return epilogue

            def out_unit(qc, i):
                """One eighth of qc's out-projection: 128 tokens x 512
                out-features -> y_parts[qc]."""
                def fn():
                    t2 = qc * 4 + i // 2
                    ec = i % 2
                    pot = pmid.tile([128, TOKCH], F32, tag="m", name="pot")
                    for fc in range(4):
                        nc.tensor.matmul(
                            pot,
                            lhsT=yt_sb[:, fc, t2 * 128 : (t2 + 1) * 128],
                            rhs=wo_sb[:, fc, ec * TOKCH : (ec + 1) * TOKCH],
                            start=(fc == 0),
                            stop=(fc == 3),
                        )
                    osb = outp.tile([128, TOKCH], BF16, tag="o", name="osb")
                    nc.vector.tensor_copy(osb, pot)
                    r2 = t2 * 128 - qc * TOKCH
                    nc.sync.dma_start(
                        out=y_parts[qc].ap()[
                            r2 : r2 + 128, ec * TOKCH : (ec + 1) * TOKCH
                        ],
                        in_=osb,
                    )
                return fn

            def out_finish(qc):
                def fn():
                    nc.gpsimd.collective_compute(
                        "ReduceScatter",
                        mybir.AluOpType.add,
                        ins=[y_parts[qc].ap().opt()],
                        outs=[rs_outs[qc].ap().opt()],
                        replica_groups=[[0, 1], [2, 3], [4, 5], [6, 7]],
                    )
                    for half2 in range(2):
                        for ec2 in range(2):
                            rt = finp.tile([128, TOKCH], BF16, tag="rt", name="rt")
                            nc.gpsimd.dma_start(
                                out=rt,
                                in_=rs_outs[qc].ap()[
                                    half2 * 128 : (half2 + 1) * 128,
                                    ec2 * TOKCH : (ec2 + 1) * TOKCH,
                                ],
                            )
                            ro = finp.tile([128, TOKCH], F32, tag="ro", name="ro")
                            nc.vector.tensor_copy(ro, rt)
                            r0 = qc * 256 + half2 * 128
                            nc.gpsimd.dma_start(
                                out=p_out[
                                    r0 : r0 + 128, ec2 * TOKCH : (ec2 + 1) * TOKCH
                                ],
                                in_=ro,
                            )
                return fn

            # ---------------- emission schedule --------------------------
            for w in range(WAVES):
                for t in range(NT):
                    emit_P_chunk(w, t)

            filler = []   # deferred out-proj units from the previous qc
            prev_epi = None
            for qc in range(NT):
                for w in range(WAVES):
                    prologue = []
                    if prev_epi is not None:
                        prologue.append(prev_epi)
                        prev_epi = None
                    # spread up to 3 out-proj filler units per wave
                    for _ in range(3):
                        if filler:
                            prologue.append(filler.pop(0))
                    prev_epi = emit_D(qc, w, prologue)
                # epilogue of wave 3 must land before qc's out-proj units
                assert not filler
                filler = [out_unit(qc, i) for i in range(8)] + [out_finish(qc)]
                if qc == NT - 1:
                    prev_epi()
                    for fn in filler:
                        fn()
                    filler = []

    nc.compile()
    return nc


def _host_prep(x, mask, pos, W_qkv, W_out, qn_w, kn_w):
    x = np.asarray(x, dtype=np.float32)
    mask = np.asarray(mask)
    pos = np.asarray(pos).astype(np.float64)
    W_qkv = np.asarray(W_qkv, dtype=np.float32)
    W_out = np.asarray(W_out, dtype=np.float32)
    qn_w = np.asarray(qn_w, dtype=np.float32)
    kn_w = np.asarray(kn_w, dtype=np.float32)

    inv_freq = 1.0 / (ROPE_BASE ** (np.arange(0, D_HEAD, 2, dtype=np.float64) / D_HEAD))
    ang = pos[:, None] * inv_freq[None, :]  # (N, 32)
    cosT = np.cos(ang).T.astype(np.float32)  # (32, N)
    sinT = np.sin(ang).T.astype(np.float32)

    # gain-free tables shared by q and k (gains applied via per-partition
    # ACT scale on the raw copies)
    cos_d = np.tile(cosT, (4, 1))
    sin_d = np.tile(np.concatenate([-sinT, sinT], axis=0), (2, 1))
    rope = np.stack([cos_d, sin_d], axis=1).astype(BF)  # (128, 2, N)
    wcol_np = np.stack([np.tile(qn_w, 2), np.tile(kn_w, 2)], axis=1).astype(
        np.float32
    )  # (128, 2)

    pswap_np = np.zeros((128, 128), dtype=np.float32)
    for a in range(2):
        for r in range(32):
            pswap_np[64 * a + r, 64 * a + 32 + r] = 1.0
            pswap_np[64 * a + 32 + r, 64 * a + r] = 1.0
    pswap_np = pswap_np.astype(BF)

    ind2_np = np.zeros((128, 2), dtype=np.float32)
    ind2_np[0:64, 0] = 1.0
    ind2_np[64:128, 1] = 1.0
    ind2_np = ind2_np.astype(BF)
    wfold_np = np.ascontiguousarray(ind2_np.T)  # (2, 128)

    state, patterns = _classify_mask(mask)
    if patterns:
        pat = np.stack(patterns, axis=1).astype(BF)
    else:
        pat = None

    q_rows = lambda h: slice(h * 192, h * 192 + 64)
    k_rows = lambda h: slice(h * 192 + 64, h * 192 + 128)
    v_rows = lambda h: slice(h * 192 + 128, h * 192 + 192)

    in_maps = []
    for c in range(N_CORES):
        b, half = divmod(c, 2)
        hs = [8 * half + i for i in range(8)]
        wqk = np.concatenate(
            [W_qkv[q_rows(h)] for h in hs] + [W_qkv[k_rows(h)] for h in hs], axis=0
        ).T  # (1024 dmodel, 1024 cols)
        wv = np.concatenate([W_qkv[v_rows(h)] for h in hs], axis=0).T
        wo = W_out[:, 512 * half : 512 * half + 512].T  # (512, 1024)
        # (128, 2, WAVES, NDC, 128): [p, qk, w, dc, f]
        wqk_re = np.ascontiguousarray(
            wqk.reshape(NDC, 128, 2, WAVES, 128).transpose(1, 2, 3, 0, 4)
        )
        wv_re = np.ascontiguousarray(
            wv.reshape(NDC, 128, WAVES, 128).transpose(1, 2, 0, 3)
        )
        wo_re = np.ascontiguousarray(wo.reshape(4, 128, 1024).transpose(1, 0, 2))
        m = {
            "xt": np.ascontiguousarray(x[b].T).astype(BF),
            "wqk": wqk_re.astype(BF),
            "wv": wv_re.astype(BF),
            "wo": wo_re.astype(BF),
            "rope": rope,
            "wcol": wcol_np,
            "ind2": ind2_np,
            "wfold": wfold_np,
            "pswap": pswap_np,
        }
        if pat is not None:
            m["pat"] = pat
        in_maps.append(m)
    return in_maps, state, (0 if pat is None else pat.shape[1])


def kernel(x, mask, pos, W_qkv, W_out, qn_w, kn_w, _trace=False):
    in_maps, state, n_pat = _host_prep(x, mask, pos, W_qkv, W_out, qn_w, kn_w)
    key = (str(state), n_pat)
    if key not in _CACHE:
        _CACHE[key] = _build_program(state, n_pat)
    nc = _CACHE[key]
    res = run_bass_kernel_spmd(nc, in_maps, list(range(N_CORES)), trace=_trace)
    out = np.empty((B, N, D_MODEL), dtype=np.float32)
    for b in range(B):
        lo = res.results[2 * b]["out"]
        hi = res.results[2 * b + 1]["out"]
        for qc in range(NT):
            out[b, qc * TOKCH : qc * TOKCH + 256] = lo[qc * 256 : (qc + 1) * 256]
            out[b, qc * TOKCH + 256 : (qc + 1) * TOKCH] = hi[qc * 256 : (qc + 1) * 256]
    kernel._last_results = res
    return out


# revision 30
# speedup vs baseline: 1.2793x; 1.2255x over previous
"""Trainium2 Bass kernel for nn_MHA_63118839382398.

Full MHA block: fused QKV projection, per-head RMSNorm on q/k, rotate-half
RoPE, causal softmax attention, output projection.

Sharding over 8 NeuronCores: core c handles batch b = c//2 and heads
[8*(c%2), 8*(c%2)+8) (tensor parallel over head halves within a batch
pair). Each core computes a partial out-projection; a 2-rank
ReduceScatter (bf16) over each pair sums the partials and leaves each
core with half of that batch's token rows, which the host reassembles.

Layout strategy (all transposed, feats x tokens), so every matmul
contraction sits on the partition axis with no on-chip transposes except
V (cheap PE-mode 128x128 transposes).

v2 structure (two phases, engine-balance driven):
  P phase: projections + rms + rope for ALL 4 head-pair waves, emitted
    densely so the PE never idles (HAM stays warm). Copies ride the
    otherwise-idle ACT engine; rms/rec broadcasts ride gpsimd
    partition_broadcast.
  A phase: attention in query-chunk-major order (qc outer, wave inner):
    scores for both head halves go to one 2-bank PSUM tile so ONE exp
    instruction covers both (ACT fixed cost ~430ns dominates, so batch).
    Each qc's out-projection + 2-rank ReduceScatter is spread into the
    next qc's attention as PE filler, so collectives overlap compute.
    Epilogue 1/denominator on DVE reciprocal (no ACT Ln/Exp).
"""

import sys

if "/opt/trn_rl_repo" not in sys.path:
    sys.path.insert(0, "/opt/trn_rl_repo")

import numpy as np
import ml_dtypes

import concourse.bass as bass
import concourse.tile as tile
from concourse import bacc, mybir
from concourse.bass_utils import run_bass_kernel_spmd
from concourse.masks import make_identity

# Problem constants (hardcoded per harness contract).
B = 4
N = 2048
D_MODEL = 1024
N_HEADS = 16
D_HEAD = 64
ROPE_BASE = 10000.0
EPS = float(np.finfo(np.float32).eps)
N_CORES = 8

HPC = N_HEADS // 2          # heads per core = 8
WAVES = HPC // 2            # head-pair waves = 4
TOKCH = 512                 # token chunk for projections / q chunks
NT = N // TOKCH             # 4
QT = 128                    # query tile for mask classification
NQT = N // QT               # 16
KB = 128                    # key block
NKB = N // KB               # 16
DC = 128                    # dmodel chunk
NDC = D_MODEL // DC         # 8

F32 = mybir.dt.float32
BF16 = mybir.dt.bfloat16
BF = ml_dtypes.bfloat16

ACT = mybir.ActivationFunctionType

_CACHE = {}


def _pin_act_tables(arch):
    """Steer bacc's ACT-table-set choice to natural_log_exp_and_others.

    The insertion pass picks the first set containing each activation's
    function; removing our functions from every other set's selection
    metadata makes Copy/Square/Ln/Exp resolve to the one set that has
    them all (no mid-kernel table reloads)."""
    from concourse.hw_specs import get_activation_tables

    tables = get_activation_tables(arch)  # cached by reference
    keep = "natural_log_exp_and_others"
    if keep not in tables:
        return
    ours = {ACT.Copy, ACT.Square, ACT.Ln, ACT.Exp, ACT.Identity}
    for name, fns in tables.items():
        if name != keep:
            fns -= ours


def _classify_mask(mask):
    """Per (key-block, query-tile) classification of the mask.

    Returns (state[NKB][NQT], patterns): state is 'skip' (all masked),
    'full' (none masked), or an index into patterns: unique (128,128)
    bf16 0/1 tiles indexed [key, query]."""
    mask = np.asarray(mask)
    assert mask.shape == (N, N)
    patterns = []
    pat_keys = {}
    state = [[None] * NQT for _ in range(NKB)]
    for kb in range(NKB):
        for qt in range(NQT):
            blk = mask[qt * QT : (qt + 1) * QT, kb * KB : (kb + 1) * KB]
            if blk.all():
                state[kb][qt] = "skip"
            elif not blk.any():
                state[kb][qt] = "full"
            else:
                tileq = (~blk.T).astype(BF)
                key = tileq.tobytes()
                if key not in pat_keys:
                    pat_keys[key] = len(patterns)
                    patterns.append(tileq)
                state[kb][qt] = pat_keys[key]
    return state, patterns


def _build_program(state, n_patterns):
    """Build the SPMD Bass program (same graph on all 8 cores)."""
    nc = bacc.Bacc(
        "TRN2", target_bir_lowering=False, debug=False, num_devices=N_CORES
    )
    _pin_act_tables(nc.m.arch)

    p_xt = nc.dram_tensor("xt", [D_MODEL, N], BF16, kind="ExternalInput").ap()
    p_wqk = nc.dram_tensor("wqk", [128, 2, WAVES, NDC, 128], BF16, kind="ExternalInput").ap()
    p_wv = nc.dram_tensor("wv", [128, WAVES, NDC, 128], BF16, kind="ExternalInput").ap()
    p_wo = nc.dram_tensor("wo", [128, 4, D_MODEL], BF16, kind="ExternalInput").ap()
    # rope tables (gain-free, shared by q/k): [128, cos/sin, N]
    p_rope = nc.dram_tensor("rope", [128, 2, N], BF16, kind="ExternalInput").ap()
    p_wcol = nc.dram_tensor("wcol", [128, 2], F32, kind="ExternalInput").ap()
    p_ind2 = nc.dram_tensor("ind2", [128, 2], BF16, kind="ExternalInput").ap()
    p_wfold = nc.dram_tensor("wfold", [2, 128], BF16, kind="ExternalInput").ap()
    p_pswap = nc.dram_tensor("pswap", [128, 128], BF16, kind="ExternalInput").ap()
    if n_patterns:
        p_pat = nc.dram_tensor(
            "pat", [128, n_patterns, 128], BF16, kind="ExternalInput"
        ).ap()
    p_out = nc.dram_tensor("out", [N // 2, D_MODEL], F32, kind="ExternalOutput").ap()

    y_parts = [
        nc.dram_tensor(f"y_part{qc}", [TOKCH, D_MODEL], BF16) for qc in range(NT)
    ]
    rs_outs = [
        nc.dram_tensor(f"rs_out{qc}", [256, D_MODEL], BF16) for qc in range(NT)
    ]
    # tiny dummy collective to absorb the ~11.5us cc-stream spin-up
    cc_warm_in = nc.dram_tensor("ccw_in", [2, 128], BF16)
    cc_warm_out = nc.dram_tensor("ccw_out", [1, 128], BF16)

    QPC = TOKCH // QT  # query tiles per chunk = 4
    n_kb = [0] * NT
    qlo_t = {}
    for qc in range(NT):
        for kb in range(NKB):
            sub = [state[kb][qc * QPC + j] for j in range(QPC)]
            if all(s == "skip" for s in sub):
                continue
            n_kb[qc] = max(n_kb[qc], kb + 1)
            lead = 0
            while sub[lead] == "skip":
                lead += 1
            qlo_t[(qc, kb)] = lead

    with tile.TileContext(nc) as tc:
        import contextlib

        ctx = contextlib.ExitStack()
        with ctx:
            singles = ctx.enter_context(tc.tile_pool(name="singles", bufs=1))
            wavep = ctx.enter_context(tc.tile_pool(name="wavep", bufs=2))
            invp = ctx.enter_context(tc.tile_pool(name="invp", bufs=2))
            work = ctx.enter_context(tc.tile_pool(name="work", bufs=2))
            espool = ctx.enter_context(tc.tile_pool(name="es", bufs=4))
            epi = ctx.enter_context(tc.tile_pool(name="epi", bufs=1))
            outp = ctx.enter_context(tc.tile_pool(name="outp", bufs=2))
            finp = ctx.enter_context(tc.tile_pool(name="finp", bufs=1))

            # PSUM budget (8 banks): ps tag "s" 3x[128,2,512]f32 = 6 banks,
            # po 1x[128,2,512] = 2 banks.
            ps = ctx.enter_context(tc.tile_pool(name="ps", bufs=3, space="PSUM"))
            ppo = ctx.enter_context(tc.tile_pool(name="ppo", bufs=1, space="PSUM"))

            # ---- resident constants -------------------------------------
            xt_sb = [singles.tile([128, NDC, TOKCH], BF16, name=f"xt{t}") for t in range(NT)]
            wqk_sb = [
                singles.tile([128, 2, NDC, 128], BF16, name=f"wqk{w}")
                for w in range(WAVES)
            ]
            wv_sb = [
                singles.tile([128, NDC, 128], BF16, name=f"wv{w}")
                for w in range(WAVES)
            ]
            rope_sb = singles.tile([128, 2, N], BF16)
            wcol = singles.tile([128, 2], F32)
            nc.scalar.dma_start(out=wcol, in_=p_wcol)
            ident = singles.tile([128, 128], BF16)
            make_identity(nc, ident)
            eps_sb = singles.tile([128, 1], F32)
            nc.vector.memset(eps_sb, EPS)
            pswap = singles.tile([128, 128], BF16)
            nc.scalar.dma_start(out=pswap, in_=p_pswap)
            ind2 = singles.tile([128, 2], BF16)
            nc.scalar.dma_start(out=ind2, in_=p_ind2)
            wfold = singles.tile([2, 128], BF16)
            nc.scalar.dma_start(out=wfold, in_=p_wfold)
            if n_patterns:
                pat_sb = singles.tile([128, n_patterns, 128], BF16)
            yt_sb = singles.tile([128, WAVES, N], BF16)
            wo_sb = singles.tile([128, 4, D_MODEL], BF16)
            # per-wave persistent attention operands
            qk_rot = [
                singles.tile([128, 2, N], BF16, name=f"qkrot{w}")
                for w in range(WAVES)
            ]
            v_sb = [
                singles.tile([128, NKB, 130], BF16, name=f"vsb{w}")
                for w in range(WAVES)
            ]

            # ---- initial DMAs: first-needed first on each queue ----------
            # sync: w0 weights + x chunk 0 (the critical path), then chunk 3
            nc.sync.dma_start(out=wqk_sb[0], in_=p_wqk[:, :, 0, :, :])
            nc.sync.dma_start(out=wv_sb[0], in_=p_wv[:, 0, :, :])
            for dc in range(NDC):
                nc.sync.dma_start(
                    out=xt_sb[0][:, dc, :], in_=p_xt[dc * DC : (dc + 1) * DC, 0:TOKCH]
                )
            # scalar: x chunk 1, w1 weights, x chunk 3
            for dc in range(NDC):
                nc.scalar.dma_start(
                    out=xt_sb[1][:, dc, :],
                    in_=p_xt[dc * DC : (dc + 1) * DC, TOKCH : 2 * TOKCH],
                )
            nc.scalar.dma_start(out=wqk_sb[1], in_=p_wqk[:, :, 1, :, :])
            for dc in range(NDC):
                nc.scalar.dma_start(
                    out=xt_sb[3][:, dc, :],
                    in_=p_xt[dc * DC : (dc + 1) * DC, 3 * TOKCH : 4 * TOKCH],
                )
            nc.scalar.dma_start(out=wqk_sb[3], in_=p_wqk[:, :, 3, :, :])
            nc.scalar.dma_start(out=wv_sb[3], in_=p_wv[:, 3, :, :])
            # gpsimd: rope + x chunk 2 + remaining weights + tables
            nc.gpsimd.dma_start(out=rope_sb, in_=p_rope)
            for dc in range(NDC):
                nc.gpsimd.dma_start(
                    out=xt_sb[2][:, dc, :],
                    in_=p_xt[dc * DC : (dc + 1) * DC, 2 * TOKCH : 3 * TOKCH],
                )
            nc.gpsimd.dma_start(out=wv_sb[1], in_=p_wv[:, 1, :, :])
            nc.gpsimd.dma_start(out=wqk_sb[2], in_=p_wqk[:, :, 2, :, :])
            nc.gpsimd.dma_start(out=wv_sb[2], in_=p_wv[:, 2, :, :])
            if n_patterns:
                nc.gpsimd.dma_start(out=pat_sb, in_=p_pat)
            nc.gpsimd.dma_start(out=wo_sb, in_=p_wo)
            for w in range(WAVES):
                nc.vector.memset(v_sb[w][:, :, 64:65], 1.0)
                nc.vector.memset(v_sb[w][:, :, 129:130], 1.0)
            # collective stream warm-up
            nc.gpsimd.collective_compute(
                "ReduceScatter",
                mybir.AluOpType.add,
                ins=[cc_warm_in.ap().opt()],
                outs=[cc_warm_out.ap().opt()],
                replica_groups=[[0, 1], [2, 3], [4, 5], [6, 7]],
            )

            # =============== P phase: proj + rms + rope ==================
            # Split into proj part (pure PE streaming) and tail part (the
            # rms/rope dependency chain); tail(c) is emitted after
            # proj(c+1) so its ACT/DVE latency hides under the next
            # chunk's matmul stream.
            def emit_P_proj(w, t):
                pj = ps.tile([128, 2, TOKCH], F32, tag="s", name="pj")
                for qk in range(2):
                    for dc in range(NDC):
                        nc.tensor.matmul(
                            pj[:, qk, :],
                            lhsT=wqk_sb[w][:, qk, dc, :],
                            rhs=xt_sb[t][:, dc, :],
                            start=(dc == 0),
                            stop=(dc == NDC - 1),
                        )
                pjv = ps.tile([128, 2, TOKCH], F32, tag="s", name="pjv")
                for dc in range(NDC):
                    nc.tensor.matmul(
                        pjv[:, 0, :],
                        lhsT=wv_sb[w][:, dc, :],
                        rhs=xt_sb[t][:, dc, :],
                        start=(dc == 0),
                        stop=(dc == NDC - 1),
                    )
                return pj, pjv

            def emit_P_tail(w, t, pj, pjv):
                tsl = slice(t * TOKCH, (t + 1) * TOKCH)
                # psum evacuation: rms gains ride the per-partition scalar
                # of DVE tensor_scalar; squares on ACT (reads psum once)
                raw = wavep.tile([128, 2, TOKCH], BF16, tag="raw", name="raw")
                for qk in range(2):
                    nc.vector.tensor_scalar_mul(
                        raw[:, qk, :], pj[:, qk, :], wcol[:, qk : qk + 1]
                    )
                sq = work.tile([128, 2, TOKCH], BF16, tag="sq")
                nc.scalar.square(sq, pj)          # ACT (pre-gain squares)
                vt = work.tile([128, TOKCH], BF16, tag="vt")
                nc.scalar.copy(vt, pjv[:, 0, :])  # ACT
                lnm = work.tile([2, 2, TOKCH], BF16, tag="lnm")
                inv = invp.tile([2, 2, TOKCH], BF16, tag="inv", name="inv")
                ssp = ps.tile([2, 2, TOKCH], F32, tag="s", name="ssp")
                for qk in range(2):
                    nc.tensor.matmul(
                        ssp[:, qk, :], lhsT=ind2, rhs=sq[:, qk, :],
                        start=True, stop=True,
                    )
                    nc.scalar.activation(
                        lnm[:, qk, :], ssp[:, qk, :], ACT.Ln,
                        bias=eps_sb[0:2, :], scale=1.0 / D_HEAD,
                    )
                    nc.scalar.activation(
                        inv[:, qk, :], lnm[:, qk, :], ACT.Exp, scale=-0.5
                    )
                # V transposes into (keys x dims) layout
                ptr = ps.tile([128, 4, 128], BF16, tag="s", name="ptr")
                for sview in range(4):
                    nc.tensor.transpose(
                        ptr[:, sview, :],
                        vt[:, sview * 128 : (sview + 1) * 128],
                        ident,
                    )
                kb0 = t * 4
                nc.vector.tensor_copy(
                    v_sb[w][:, kb0 : kb0 + 4, 0:64], ptr[:, :, 0:64]
                )
                nc.vector.tensor_copy(
                    v_sb[w][:, kb0 : kb0 + 4, 65:129], ptr[:, :, 64:128]
                )
                # rope: fac broadcast + rotate-half swap per qk in one tile
                qn = work.tile([128, 2, TOKCH], BF16, tag="qn")
                qcos = work.tile([128, 2, TOKCH], BF16, tag="qcos")
                qsin = work.tile([128, 2, TOKCH], BF16, tag="qsin")
                fsw = [None, None]
                for qk in range(2):
                    fsw[qk] = ps.tile([128, 2, TOKCH], F32, tag="s", name="fsw")
                    nc.tensor.matmul(
                        fsw[qk][:, 0, :], lhsT=wfold, rhs=inv[:, qk, :],
                        start=True, stop=True,
                    )
                    nc.vector.tensor_mul(
                        qn[:, qk, :], raw[:, qk, :], fsw[qk][:, 0, :]
                    )
                for qk in range(2):
                    nc.tensor.matmul(
                        fsw[qk][:, 1, :], lhsT=pswap, rhs=qn[:, qk, :],
                        start=True, stop=True,
                    )
                    nc.vector.tensor_mul(
                        qcos[:, qk, :], qn[:, qk, :], rope_sb[:, 0, tsl]
                    )
                    nc.vector.tensor_mul(
                        qsin[:, qk, :], fsw[qk][:, 1, :], rope_sb[:, 1, tsl]
                    )
                nc.vector.tensor_add(qk_rot[w][:, :, tsl], qcos, qsin)

            # =============== A phase: attention, qc-major ================
            def emit_D(qc, w, prologue):
                """Attention for (qc, w). `prologue` is a list of closures
                (previous wave's epilogue, out-proj filler units) emitted
                between the first score pairs and the first PV so the PE
                FIFO never stalls on their dependencies. Returns this
                wave's epilogue closure."""
                kbs = [kb for kb in range(n_kb[qc]) if (qc, kb) in qlo_t]
                po = ppo.tile([128, 2, TOKCH], F32, tag="po", name="po")
                first = [True, True]
                pend = []

                def flush_pv(kb, es, last):
                    qlo = qlo_t[(qc, kb)] * QT
                    osl = slice(qlo, TOKCH)
                    for h2 in range(2):
                        nc.tensor.matmul(
                            po[0:65, h2, osl],
                            lhsT=v_sb[w][:, kb, 65 * h2 : 65 * h2 + 65],
                            rhs=es[:, h2, osl],
                            start=first[h2],
                            stop=last,
                        )
                        first[h2] = False

                for i, kb in enumerate(kbs):
                    qlo = qlo_t[(qc, kb)] * QT
                    csl = slice(qc * TOKCH + qlo, (qc + 1) * TOKCH)
                    osl = slice(qlo, TOKCH)
                    pst = ps.tile([128, 2, TOKCH], F32, tag="s", name="pst")
                    for h2 in range(2):
                        hr = slice(64 * h2, 64 * h2 + 64)
                        nc.tensor.matmul(
                            pst[:, h2, osl],
                            lhsT=qk_rot[w][hr, 1, kb * KB : (kb + 1) * KB],
                            rhs=qk_rot[w][hr, 0, csl],
                            start=True,
                            stop=True,
                        )
                    es = espool.tile([128, 2, TOKCH], BF16, tag="es", name="es")
                    nc.scalar.activation(
                        es[:, :, osl], pst[:, :, osl], ACT.Exp,
                        scale=float(D_HEAD) ** -0.5,
                    )
                    for j in range(qlo // QT, QPC):
                        st = state[kb][qc * QPC + j]
                        if isinstance(st, int):
                            jsl = slice(j * QT, (j + 1) * QT)
                            for h2 in range(2):
                                nc.vector.tensor_mul(
                                    es[:, h2, jsl], es[:, h2, jsl],
                                    pat_sb[:, st, :],
                                )
                    if prologue:
                        prologue.pop(0)()
                    pend.append((kb, es))
                    if len(pend) > 2:
                        k0, e0 = pend.pop(0)
                        flush_pv(k0, e0, False)
                for fn in prologue:
                    fn()
                for i, (k0, e0) in enumerate(pend):
                    flush_pv(k0, e0, i == len(pend) - 1)

                def epilogue():
                    # po rows 0:63 = y_raw, row 64 = softmax denominator;
                    # 1/den = exp(-ln(den)) on ACT (single-partition DVE
                    # reciprocal measured 6.5us -- ACT is flat ~650ns)
                    yr = epi.tile([64, 2, TOKCH], BF16, tag="yr", name="yr")
                    nc.vector.tensor_copy(yr, po[0:64, :, :])
                    lnd = epi.tile([1, 2, TOKCH], F32, tag="lnd", name="lnd")
                    nc.scalar.activation(lnd, po[64:65, :, :], ACT.Ln)
                    recb = epi.tile([1, 2, TOKCH], BF16, tag="recb", name="recb")
                    nc.scalar.activation(recb, lnd, ACT.Exp, scale=-1.0)
                    f2 = epi.tile([64, 2, TOKCH], BF16, tag="f2", name="f2")
                    for h2 in range(2):
                        nc.gpsimd.partition_broadcast(
                            f2[:, h2, :], recb[:, h2, :]
                        )
                    for h2 in range(2):
                        nc.vector.tensor_mul(
                            yt_sb[
                                64 * h2 : 64 * h2 + 64, w,
                                qc * TOKCH : (qc + 1) * TOKCH,
                            ],
                            yr[:, h2, :],
                            f2[:, h2, :],
                        )

                return epilogue

            def out_unit(qc, i):
                """One quarter of qc's out-projection: 128 tokens x 1024
                out-features -> y_parts[qc]."""
                def fn():
                    t2 = qc * 4 + i
                    pot = ps.tile([128, 2, TOKCH], F32, tag="s", name="pot")
                    for ec in range(2):
                        for fc in range(4):
                            nc.tensor.matmul(
                                pot[:, ec, :],
                                lhsT=yt_sb[:, fc, t2 * 128 : (t2 + 1) * 128],
                                rhs=wo_sb[:, fc, ec * TOKCH : (ec + 1) * TOKCH],
                                start=(fc == 0),
                                stop=(fc == 3),
                            )
                    osb = outp.tile([128, 2, TOKCH], BF16, tag="o", name="osb")
                    nc.vector.tensor_copy(osb, pot)
                    r2 = t2 * 128 - qc * TOKCH
                    nc.sync.dma_start(
                        out=y_parts[qc].ap()[r2 : r2 + 128, :], in_=osb
                    )
                return fn

            def rs_trigger(qc):
                def fn():
                    nc.gpsimd.collective_compute(
                        "ReduceScatter",
                        mybir.AluOpType.add,
                        ins=[y_parts[qc].ap().opt()],
                        outs=[rs_outs[qc].ap().opt()],
                        replica_groups=[[0, 1], [2, 3], [4, 5], [6, 7]],
                    )
                return fn

            def unpack(qc):
                """Post-RS f32 unpack; DMAs ride the sync queue (the only
                queue whose blocking on the RS semaphore is harmless)."""
                def fn():
                    for half2 in range(2):
                        rt = finp.tile([128, D_MODEL], BF16, tag="rt", name="rt")
                        nc.sync.dma_start(
                            out=rt,
                            in_=rs_outs[qc].ap()[half2 * 128 : (half2 + 1) * 128, :],
                        )
                        ro = finp.tile([128, D_MODEL], F32, tag="ro", name="ro")
                        nc.vector.tensor_copy(ro, rt)
                        r0 = qc * 256 + half2 * 128
                        nc.sync.dma_start(out=p_out[r0 : r0 + 128, :], in_=ro)
                return fn

            # ---------------- emission schedule --------------------------
            # P phase, software-pipelined: tail(c) emitted after proj(c+1)
            pend_tail = None
            for w in range(WAVES):
                for t in range(NT):
                    pj, pjv = emit_P_proj(w, t)
                    if pend_tail is not None:
                        emit_P_tail(*pend_tail)
                    pend_tail = (w, t, pj, pjv)
            emit_P_tail(*pend_tail)

            # A phase: out-proj of qc spreads into qc+1's waves as filler
            filler = []
            prev_epi = None
            for qc in range(NT):
                for w in range(WAVES):
                    prologue = []
                    if prev_epi is not None:
                        prologue.append(prev_epi)
                        prev_epi = None
                    for _ in range(2):
                        if filler:
                            prologue.append(filler.pop(0))
                    prev_epi = emit_D(qc, w, prologue)
                assert not filler, f"filler left over at qc={qc}"
                filler = [out_unit(qc, i) for i in range(4)] + [rs_trigger(qc)]
                if qc >= 1:
                    filler.append(unpack(qc - 1))
                if qc == NT - 1:
                    prev_epi()
                    for fn in filler:
                        fn()
                    filler = []
                    unpack(NT - 1)()

    nc.compile()
    return nc


def _host_prep(x, mask, pos, W_qkv, W_out, qn_w, kn_w):
    x = np.asarray(x, dtype=np.float32)
    mask = np.asarray(mask)
    pos = np.asarray(pos).astype(np.float64)
    W_qkv = np.asarray(W_qkv, dtype=np.float32)
    W_out = np.asarray(W_out, dtype=np.float32)
    qn_w = np.asarray(qn_w, dtype=np.float32)
    kn_w = np.asarray(kn_w, dtype=np.float32)

    inv_freq = 1.0 / (ROPE_BASE ** (np.arange(0, D_HEAD, 2, dtype=np.float64) / D_HEAD))
    ang = pos[:, None] * inv_freq[None, :]  # (N, 32)
    cosT = np.cos(ang).T.astype(np.float32)  # (32, N)
    sinT = np.sin(ang).T.astype(np.float32)

    # gain-free tables shared by q and k (gains applied via per-partition
    # ACT scale on the raw copies)
    cos_d = np.tile(cosT, (4, 1))
    sin_d = np.tile(np.concatenate([-sinT, sinT], axis=0), (2, 1))
    rope = np.stack([cos_d, sin_d], axis=1).astype(BF)  # (128, 2, N)
    wcol_np = np.stack([np.tile(qn_w, 2), np.tile(kn_w, 2)], axis=1).astype(
        np.float32
    )  # (128, 2)

    pswap_np = np.zeros((128, 128), dtype=np.float32)
    for a in range(2):
        for r in range(32):
            pswap_np[64 * a + r, 64 * a + 32 + r] = 1.0
            pswap_np[64 * a + 32 + r, 64 * a + r] = 1.0
    pswap_np = pswap_np.astype(BF)

    ind2_np = np.zeros((128, 2), dtype=np.float32)
    ind2_np[0:64, 0] = 1.0
    ind2_np[64:128, 1] = 1.0
    ind2_np = ind2_np.astype(BF)
    wfold_np = np.ascontiguousarray(ind2_np.T)  # (2, 128)

    state, patterns = _classify_mask(mask)
    if patterns:
        pat = np.stack(patterns, axis=1).astype(BF)
    else:
        pat = None

    q_rows = lambda h: slice(h * 192, h * 192 + 64)
    k_rows = lambda h: slice(h * 192 + 64, h * 192 + 128)
    v_rows = lambda h: slice(h * 192 + 128, h * 192 + 192)

    in_maps = []
    for c in range(N_CORES):
        b, half = divmod(c, 2)
        hs = [8 * half + i for i in range(8)]
        wqk = np.concatenate(
            [W_qkv[q_rows(h)] for h in hs] + [W_qkv[k_rows(h)] for h in hs], axis=0
        ).T  # (1024 dmodel, 1024 cols)
        wv = np.concatenate([W_qkv[v_rows(h)] for h in hs], axis=0).T
        wo = W_out[:, 512 * half : 512 * half + 512].T  # (512, 1024)
        # (128, 2, WAVES, NDC, 128): [p, qk, w, dc, f]
        wqk_re = np.ascontiguousarray(
            wqk.reshape(NDC, 128, 2, WAVES, 128).transpose(1, 2, 3, 0, 4)
        )
        wv_re = np.ascontiguousarray(
            wv.reshape(NDC, 128, WAVES, 128).transpose(1, 2, 0, 3)
        )
        wo_re = np.ascontiguousarray(wo.reshape(4, 128, 1024).transpose(1, 0, 2))
        m = {
            "xt": np.ascontiguousarray(x[b].T).astype(BF),
            "wqk": wqk_re.astype(BF),
            "wv": wv_re.astype(BF),
            "wo": wo_re.astype(BF),
            "rope": rope,
            "wcol": wcol_np,
            "ind2": ind2_np,
            "wfold": wfold_np,
            "pswap": pswap_np,
        }
        if pat is not None:
            m["pat"] = pat
        in_maps.append(m)
    return in_maps, state, (0 if pat is None else pat.shape[1])


def kernel(x, mask, pos, W_qkv, W_out, qn_w, kn_w, _trace=False):
    in_maps, state, n_pat = _host_prep(x, mask, pos, W_qkv, W_out, qn_w, kn_w)
    key = (str(state), n_pat)
    if key not in _CACHE:
        _CACHE[key] = _build_program(state, n_pat)
    nc = _CACHE[key]
    res = run_bass_kernel_spmd(nc, in_maps, list(range(N_CORES)), trace=_trace)
    out = np.empty((B, N, D_MODEL), dtype=np.float32)
    for b in range(B):
        lo = res.results[2 * b]["out"]
        hi = res.results[2 * b + 1]["out"]
        for qc in range(NT):
            out[b, qc * TOKCH : qc * TOKCH + 256] = lo[qc * 256 : (qc + 1) * 256]
            out[b, qc * TOKCH + 256 : (qc + 1) * TOKCH] = hi[qc * 256 : (qc + 1) * 256]
    kernel._last_results = res
    return out
